# revision 7
# baseline (speedup 1.0000x reference)
# ChildSum TreeLSTM layer (segment-sum message passing) on 8 Trainium2 cores.
#
# Strategy (see sharding hint): shard by contiguous parent-id ranges. Core m
# owns parents [m*6250, (m+1)*6250) and (because seg is sorted) a contiguous
# slice of the child edge list. Weights are replicated.
#
# On-device algorithm, per core, fully uniform across cores (single SPMD
# program):
#   - Parent space is split into NB=49 aligned blocks of 128 parents.
#   - Each block's children are processed in K_TILES tiles of 128 children
#     (host zero-pads every block to exactly K_TILES*128 children so the
#     instruction stream is identical on every core).
#   - Segment sums are matmuls against 0/1 selection matrices S built on
#     device from host-provided local parent offsets (off = seg - block_base):
#       S_ep[e,p] = (off[e] == p)   e on partitions  (rhs of reduce matmuls)
#       S_pe = S_ep^T via PE transpose              (lhsT of the W_f gather)
#   - Per tile:  f_pre = S_pe^T @ WxF_block + (ch^T)^T @ U_f   (PSUM accum)
#                f_k = sigmoid(f_pre); m = f_k * cc
#                hsT  += ch^T_sel:  matmul(lhsT=ch,  rhs=S_ep)  (PSUM accum)
#                bfT  += m^T_sel:   matmul(lhsT=m,   rhs=S_ep)  (PSUM accum)
#   - Per block: Wx = x_block @ W (f32r), iuo = hsT^T @ U_iuo accumulated
#     onto Wx[:,128:512] in PSUM, leaf h_init fix added as a rank-1 matmul
#     mask ⊗ (h_init @ U_iuo), then gates + outputs.
import math
import os

import numpy as np

D = 128
NCORES = 8
N_TOTAL = 50000
E_TOTAL = 800000
P_CORE = N_TOTAL // NCORES  # 6250
PB = 128  # parents per block
NB = math.ceil(P_CORE / PB)  # 49
NP_PAD = NB * PB  # 6272
PAD_OFF = 255.0  # sentinel local offset for padded children (matches nothing)


def _host_prep(x, child_h, child_c, seg):
    """Shard + pad inputs per core. Returns (per_core_list, K_TILES)."""
    seg = np.ascontiguousarray(np.asarray(seg, dtype=np.int64))
    x = np.asarray(x, dtype=np.float32)
    child_h = np.asarray(child_h, dtype=np.float32)
    child_c = np.asarray(child_c, dtype=np.float32)

    counts = np.bincount(seg, minlength=N_TOTAL)

    # block edges per core (parent ids), child boundaries per block
    all_cb = []
    max_tiles = 1
    for m in range(NCORES):
        pstart = m * P_CORE
        edges = pstart + np.minimum(np.arange(NB + 1) * PB, P_CORE)
        cb = np.searchsorted(seg, edges)
        cnts = np.diff(cb)
        max_tiles = max(max_tiles, int(np.max((cnts + 127) // 128)))
        all_cb.append(cb)
    K_TILES = int(max_tiles)
    T_CORE = NB * K_TILES
    E_PAD = T_CORE * 128

    cores = []
    for m in range(NCORES):
        pstart = m * P_CORE
        cb = all_cb[m]
        cnts = np.diff(cb)

        # destination indices for this core's (unpadded) children
        dest = np.concatenate(
            [
                np.arange(cnts[b], dtype=np.int64) + b * K_TILES * 128
                for b in range(NB)
            ]
        )
        src_lo, src_hi = cb[0], cb[-1]

        ch_pad = np.zeros((E_PAD, D), np.float32)
        cc_pad = np.zeros((E_PAD, D), np.float32)
        ch_pad[dest] = child_h[src_lo:src_hi]
        cc_pad[dest] = child_c[src_lo:src_hi]

        offs = np.full((E_PAD,), PAD_OFF, np.float32)
        block_base = np.repeat(
            pstart + np.arange(NB, dtype=np.int64) * PB, cnts
        )
        offs[dest] = (seg[src_lo:src_hi] - block_base).astype(np.float32)
        assert offs[dest].min() >= 0 and offs[dest].max() < PB
        offs = offs.reshape(T_CORE, 128)

        x_pad = np.zeros((NP_PAD, D), np.float32)
        x_pad[:P_CORE] = x[pstart : pstart + P_CORE]

        mask = np.ones((NP_PAD,), np.float32)
        mask[:P_CORE] = (counts[pstart : pstart + P_CORE] == 0).astype(
            np.float32
        )
        mask = mask.reshape(NB, PB)

        cores.append(
            {"x": x_pad, "ch": ch_pad, "cc": cc_pad, "offs": offs, "msk": mask}
        )
    return cores, K_TILES, T_CORE, E_PAD


def _build_nc(K_TILES, T_CORE, E_PAD):
    import concourse.bacc as bacc
    import concourse.mybir as mybir
    from concourse.masks import make_identity
    from concourse.tile import TileContext
    from contextlib import ExitStack

    f32 = mybir.dt.float32
    f32r = mybir.dt.float32r
    bf16 = mybir.dt.bfloat16
    AF = mybir.ActivationFunctionType
    OP = mybir.AluOpType

    nc = bacc.Bacc("TRN2", target_bir_lowering=False)

    x_d = nc.dram_tensor("x", [NP_PAD, D], f32, kind="ExternalInput")
    ch_d = nc.dram_tensor("ch", [E_PAD, D], f32, kind="ExternalInput")
    cc_d = nc.dram_tensor("cc", [E_PAD, D], f32, kind="ExternalInput")
    offs_d = nc.dram_tensor("offs", [T_CORE, 128], f32, kind="ExternalInput")
    msk_d = nc.dram_tensor("msk", [NB, PB], f32, kind="ExternalInput")
    W_d = nc.dram_tensor("W", [D, 4 * D], f32, kind="ExternalInput")
    Uf_d = nc.dram_tensor("Uf", [D, D], f32, kind="ExternalInput")
    Uiuo_d = nc.dram_tensor("Uiuo", [D, 3 * D], f32, kind="ExternalInput")
    hU_d = nc.dram_tensor("hU", [1, 3 * D], f32, kind="ExternalInput")
    outc_d = nc.dram_tensor("outc", [NP_PAD, D], f32, kind="ExternalOutput")
    outh_d = nc.dram_tensor("outh", [NP_PAD, D], f32, kind="ExternalOutput")

    with TileContext(nc) as tc, ExitStack() as ctx:
        const = ctx.enter_context(tc.tile_pool(name="const", bufs=1))

        ident_f = const.tile([128, 128], f32, tag="ident_f")
        make_identity(nc, ident_f[:])
        ident_b = const.tile([128, 128], bf16, tag="ident_b")
        make_identity(nc, ident_b[:])

        # iota over the free dim (value = free index), f32 (exact to 2^24)
        iota_row = const.tile([128, 128], f32, tag="iota_row")
        nc.gpsimd.iota(
            iota_row[:],
            [[1, 128]],
            channel_multiplier=0,
            allow_small_or_imprecise_dtypes=True,
        )

        W_sb = const.tile([D, 4 * D], f32, tag="W_sb")
        nc.sync.dma_start(W_sb[:], W_d[:])
        W_sbr = const.tile([D, 4 * D], f32r, tag="W_sbr")
        nc.vector.tensor_copy(W_sbr[:], W_sb[:])
        Uf_sb = const.tile([D, D], f32, tag="Uf_sb")
        nc.sync.dma_start(Uf_sb[:], Uf_d[:])
        Uf_bf = const.tile([D, D], bf16, tag="Uf_bf")
        nc.vector.tensor_copy(Uf_bf[:], Uf_sb[:])
        Uiuo_sb = const.tile([D, 3 * D], f32, tag="Uiuo_sb")
        nc.sync.dma_start(Uiuo_sb[:], Uiuo_d[:])
        Uiuo_r = const.tile([D, 3 * D], f32r, tag="Uiuo_r")
        nc.vector.tensor_copy(Uiuo_r[:], Uiuo_sb[:])
        hU = const.tile([1, 3 * D], f32, tag="hU")
        nc.sync.dma_start(hU[:], hU_d[:])
        hU_r = const.tile([1, 3 * D], f32r, tag="hU_r")
        nc.vector.tensor_copy(hU_r[:], hU[:])

        # pools
        xp = ctx.enter_context(tc.tile_pool(name="xp", bufs=2))
        chp = ctx.enter_context(tc.tile_pool(name="chp", bufs=4))
        ccp = ctx.enter_context(tc.tile_pool(name="ccp", bufs=4))
        offp = ctx.enter_context(tc.tile_pool(name="offp", bufs=2))
        sep = ctx.enter_context(tc.tile_pool(name="sep", bufs=3))
        sebp = ctx.enter_context(tc.tile_pool(name="sebp", bufs=3))
        spp = ctx.enter_context(tc.tile_pool(name="spp", bufs=3))
        chtp = ctx.enter_context(tc.tile_pool(name="chtp", bufs=3))
        chbfp = ctx.enter_context(tc.tile_pool(name="chbfp", bufs=3))
        fkp = ctx.enter_context(tc.tile_pool(name="fkp", bufs=3))
        mp = ctx.enter_context(tc.tile_pool(name="mp", bufs=3))
        wxfp = ctx.enter_context(tc.tile_pool(name="wxfp", bufs=2))
        hsp = ctx.enter_context(tc.tile_pool(name="hsp", bufs=2))
        gp = ctx.enter_context(tc.tile_pool(name="gp", bufs=2))
        outp = ctx.enter_context(tc.tile_pool(name="outp", bufs=2))
        mskp = ctx.enter_context(tc.tile_pool(name="mskp", bufs=2))

        tps = ctx.enter_context(tc.tile_pool(name="tps", bufs=2, space="PSUM"))
        wxps = ctx.enter_context(
            tc.tile_pool(name="wxps", bufs=2, space="PSUM")
        )
        bps = ctx.enter_context(tc.tile_pool(name="bps", bufs=2, space="PSUM"))
        rps = ctx.enter_context(tc.tile_pool(name="rps", bufs=2, space="PSUM"))

        for b in range(NB):
            # ---- block prologue: Wx = x_block @ W ----
            x_t = xp.tile([128, D], f32, tag="x_t")
            nc.sync.dma_start(x_t[:], x_d[b * PB : (b + 1) * PB, :])
            b_ps = bps.tile([128, 256], f32, tag="b_ps")
            nc.tensor.transpose(b_ps[:, 0:128], x_t[:], ident_f[:])
            xT_s = xp.tile([128, 128], f32r, tag="xT_s")
            nc.scalar.copy(xT_s[:], b_ps[:, 0:128])

            wx_ps = wxps.tile([128, 4 * D], f32, tag="wx_ps")
            nc.tensor.matmul(
                wx_ps[:],
                lhsT=xT_s[:],
                rhs=W_sbr[:],
                start=True,
                stop=False,
                skip_group_check=True,
            )
            wxf_bf = wxfp.tile([128, 128], bf16, tag="wxf_bf")
            nc.scalar.copy(wxf_bf[:], wx_ps[:, 0:128])

            # block off columns: [128 e, K_TILES]
            offB = offp.tile([128, K_TILES], f32, tag="offB")
            nc.sync.dma_start(
                offB[:],
                offs_d[b * K_TILES : (b + 1) * K_TILES, :].rearrange(
                    "k e -> e k"
                ),
            )

            r_ps = rps.tile([128, 256], f32, tag="r_ps")
            hs_ps = r_ps[:, 0:128]
            bf_ps = r_ps[:, 128:256]

            for k in range(K_TILES):
                t = b * K_TILES + k
                ch_t = chp.tile([128, D], f32, tag="ch_t")
                nc.sync.dma_start(
                    ch_t[:], ch_d[t * 128 : (t + 1) * 128, :]
                )
                cc_t = ccp.tile([128, D], f32, tag="cc_t")
                nc.sync.dma_start(
                    cc_t[:], cc_d[t * 128 : (t + 1) * 128, :]
                )

                # S_ep[e,p] = (off[e] == p)
                S_ep = sep.tile([128, 128], f32, tag="S_ep")
                nc.vector.tensor_scalar(
                    S_ep[:],
                    iota_row[:],
                    offB[:, k : k + 1],
                    None,
                    OP.is_equal,
                )
                S_ep_bf = sebp.tile([128, 128], bf16, tag="S_ep_bf")
                nc.gpsimd.tensor_copy(S_ep_bf[:], S_ep[:])
                t_ps = tps.tile([128, 384], f32, tag="t_ps")
                # S_pe = S_ep^T (PE transpose f32, copy-cast to bf16)
                nc.tensor.transpose(t_ps[:, 0:128], S_ep[:], ident_f[:])
                S_pe = spp.tile([128, 128], bf16, tag="S_pe")
                nc.scalar.copy(S_pe[:], t_ps[:, 0:128])

                # chT (bf16) via PE transpose
                nc.tensor.transpose(t_ps[:, 128:256], ch_t[:], ident_f[:])
                chT = chtp.tile([128, 128], bf16, tag="chT")
                nc.scalar.copy(chT[:], t_ps[:, 128:256])

                # ch in bf16 for the h reduce (gpsimd does the cast)
                ch_bf = chbfp.tile([128, 128], bf16, tag="ch_bf")
                nc.gpsimd.tensor_copy(ch_bf[:], ch_t[:])

                # f_pre = gather(WxF) + ch @ U_f
                f_ps = t_ps[:, 256:384]
                nc.tensor.matmul(
                    f_ps, lhsT=S_pe[:], rhs=wxf_bf[:], start=True,
                    stop=False,
                )
                nc.tensor.matmul(
                    f_ps, lhsT=chT[:], rhs=Uf_bf[:], start=False, stop=True
                )
                f_k = fkp.tile([128, 128], f32, tag="f_k")
                nc.scalar.activation(f_k[:], f_ps, AF.Sigmoid)

                m_bf = mp.tile([128, 128], bf16, tag="m_bf")
                nc.vector.tensor_mul(m_bf[:], f_k[:], cc_t[:])

                # reduces (accumulate across the block's tiles in PSUM)
                nc.tensor.matmul(
                    hs_ps,
                    lhsT=ch_bf[:],
                    rhs=S_ep_bf[:],
                    start=(k == 0),
                    stop=False,
                    skip_group_check=True,
                )
                nc.tensor.matmul(
                    bf_ps,
                    lhsT=m_bf[:],
                    rhs=S_ep_bf[:],
                    start=False,
                    stop=(k == K_TILES - 1),
                    skip_group_check=True,
                )

            # ---- block epilogue ----
            hsT_s = hsp.tile([128, 128], f32r, tag="hsT_s")
            nc.scalar.copy(hsT_s[:], hs_ps)
            nc.tensor.matmul(
                wx_ps[:, 128:512],
                lhsT=hsT_s[:],
                rhs=Uiuo_r[:],
                start=False,
                stop=False,
                skip_group_check=True,
            )
            msk_t = mskp.tile([1, 128], f32, tag="msk_t")
            nc.sync.dma_start(msk_t[:], msk_d[b : b + 1, :])
            msk_r = mskp.tile([1, 128], f32r, tag="msk_r")
            nc.vector.tensor_copy(msk_r[:], msk_t[:])
            nc.tensor.matmul(
                wx_ps[:, 128:512],
                lhsT=msk_r[:],
                rhs=hU_r[:],
                start=False,
                stop=True,
                skip_group_check=True,
            )

            bi = gp.tile([128, 128], f32, tag="bi")
            nc.scalar.activation(bi[:], wx_ps[:, 128:256], AF.Sigmoid)
            bu = gp.tile([128, 128], f32, tag="bu")
            nc.scalar.activation(bu[:], wx_ps[:, 256:384], AF.Tanh)
            bo = gp.tile([128, 128], f32, tag="bo")
            nc.scalar.activation(bo[:], wx_ps[:, 384:512], AF.Sigmoid)

            bf_sb = hsp.tile([128, 128], f32, tag="bf_sb")
            nc.scalar.copy(bf_sb[:], bf_ps)
            bfT_ps = b_ps[:, 128:256]
            nc.tensor.transpose(bfT_ps, bf_sb[:], ident_f[:])

            iu = outp.tile([128, 128], f32, tag="iu")
            nc.vector.tensor_mul(iu[:], bi[:], bu[:])
            new_c = outp.tile([128, 128], f32, tag="new_c")
            nc.vector.tensor_add(new_c[:], iu[:], bfT_ps)
            tanh_c = outp.tile([128, 128], f32, tag="tanh_c")
            nc.scalar.activation(tanh_c[:], new_c[:], AF.Tanh)
            new_h = outp.tile([128, 128], f32, tag="new_h")
            nc.vector.tensor_mul(new_h[:], bo[:], tanh_c[:])

            nc.sync.dma_start(outc_d[b * PB : (b + 1) * PB, :], new_c[:])
            nc.sync.dma_start(outh_d[b * PB : (b + 1) * PB, :], new_h[:])

    nc.compile()
    return nc


def kernel(x, child_h, child_c, seg, W, U_f, U_iuo, h_init):
    from concourse.bass_utils import run_bass_kernel_spmd

    cores, K_TILES, T_CORE, E_PAD = _host_prep(x, child_h, child_c, seg)
    nc = _build_nc(K_TILES, T_CORE, E_PAD)

    W = np.asarray(W, np.float32)
    U_f = np.asarray(U_f, np.float32)
    U_iuo = np.asarray(U_iuo, np.float32)
    h_init = np.asarray(h_init, np.float32).reshape(1, D)
    hU = (h_init @ U_iuo).astype(np.float32)

    in_maps = []
    for c in cores:
        in_maps.append(
            {
                "x": c["x"],
                "ch": c["ch"],
                "cc": c["cc"],
                "offs": c["offs"],
                "msk": c["msk"],
                "W": W,
                "Uf": U_f,
                "Uiuo": U_iuo,
                "hU": hU,
            }
        )

    res = run_bass_kernel_spmd(
        nc,
        in_maps,
        core_ids=list(range(NCORES)),
        trace=bool(int(os.environ.get("KERNEL_TRACE", "0"))),
    )
    if res.exec_time_ns is not None:
        print(f"HW exec time: {res.exec_time_ns} ns")

    new_c = np.empty((N_TOTAL, D), np.float32)
    new_h = np.empty((N_TOTAL, D), np.float32)
    for m, r in enumerate(res.results):
        new_c[m * P_CORE : (m + 1) * P_CORE] = r["outc"][:P_CORE]
        new_h[m * P_CORE : (m + 1) * P_CORE] = r["outh"][:P_CORE]
    return new_c, new_h
